# revision 18
# baseline (speedup 1.0000x reference)
"""CapsuleLayer (dynamic routing, 3 iterations) on 8 Trainium2 NeuronCores.

Decomposition (never materializes u_hat = [256,1152,10,16], 189MB):
  - Shard the 1152 input capsules (i) 8 ways: 144 per core.
  - Per-core row space j = (i_local, k), k = in_size = 8 -> 1152 rows
    = 9 chunks of 128 partitions.
  - s_j:  s[b,(n,o)] = sum_j xT[j,b] * (c[j,n] * Wl[j,(n,o)])   (PE matmul,
    contraction over j; Wl = 0.03*W in [(i,k),(n,o)] layout, c broadcast
    over k and o).  Partial over the i-shard -> summed across cores by a
    fp32 AllReduce straight out of PSUM (no SBUF copies, no on-chip tree).
  - b_ij update via a Gram matrix instead of u_hat:
       Q[j,(n,o)]  = sum_b x[b,j] * v[b,(n,o)]                  (PE matmul)
       pr[j,n]     = sum_o Wl[j,(n,o)] * Q[j,(n,o)]             (DVE)
       uv_rows     = F.T @ pr  per 128-chunk, F = kron(I16, ones8x8)/B
                     (sums over k within each i-group AND replicates the
                     result back to all k-rows, so b stays row-replicated)
  - Iteration 1 uses uniform c = 1/10 (softmax of zeros): s1 = 0.1*(xT.T@Wl).
  - Iteration 3 needs no b-update; the fp32 s3 goes through ReduceScatter
    (also straight out of PSUM) so each core squashes only its 32-row
    batch shard; the host just concatenates the 8 shards.

Latency plan (the kernel is serial-latency-bound, engines are <20% busy):
  - The 8-core rendezvous barrier releases when the LAST core triggers its
    first collective, so per-core time-to-first-trigger is on every core's
    critical path.  All inputs are host-transposed to partition-major so
    each loads with ONE plain 2D contiguous DMA (3D-strided dma_starts
    cost ~1.3-1.8us of descriptor generation each; 2D cost ~0.6us).
  - Routing matmuls use bf16 operands (fp32 PE matmuls are 4x slower).
  - squash per 128-batch-chunk is pipelined with the Q matmuls of the
    previous chunk; the W-sized mc = c*Wl multiply is split DVE/GpSimd.
  - sqrt is a bit-trick + Newton on the DVE so the ScalarE only ever needs
    the Exp activation table (Sqrt/Ln live in other table sets and would
    force ~2.7us ACT_TABLE_LOADs per iteration).  The routing squashes
    skip the Newton step entirely (~3.4% rsqrt error, which averages out
    across the 1152-capsule contraction); the output squash uses two.
"""
import sys

if "/opt/trn_rl_repo" not in sys.path:
    sys.path.insert(0, "/opt/trn_rl_repo")

import numpy as np

import os
N_CORES = int(os.environ.get("KERNEL_CORES", "8"))
B, IN_SIZE, I_TOT = 256, 8, 1152
N_NODE, O_SZ = 10, 16
NO = N_NODE * O_SZ          # 160
I_SH = I_TOT // N_CORES     # 144 capsules per core
JR = I_SH * IN_SIZE         # 1152 rows per core
NCH = JR // 128             # 9 contraction chunks
BC = B // 128               # 2 batch chunks
FCH = I_TOT * IN_SIZE // 128  # 72 full-row chunks (replicated iteration 1)
B_SH = B // N_CORES         # 32 batch rows per core after ReduceScatter

RSQRT_MAGIC = 0x5F3759DF
NEWTON_ROUTE = int(os.environ.get("KERNEL_NEWTON_ROUTE", "0"))
EXCHANGE = os.environ.get("KERNEL_EXCHANGE", "ag8")

_CACHE = {}


def _build_program():
    import concourse.bacc as bacc
    import concourse.tile as tile
    import concourse.mybir as mybir

    f32 = mybir.dt.float32
    bf16 = mybir.dt.bfloat16
    f8 = mybir.dt.float8e4
    i32 = mybir.dt.int32
    AF = mybir.ActivationFunctionType
    ALU = mybir.AluOpType
    AX = mybir.AxisListType

    nc = bacc.Bacc("TRN2", target_bir_lowering=False, debug=False,
                   enable_asserts=False, num_devices=N_CORES)

    # All inputs partition-major: one plain 2D contiguous DMA each.
    xf8_d = nc.dram_tensor("xf8", [128, FCH * B], f8,
                           kind="ExternalInput").ap()
    wf8_d = nc.dram_tensor("wf8", [128, FCH * NO], f8,
                           kind="ExternalInput").ap()
    xt_d = nc.dram_tensor("xt", [128, NCH * B], bf16,
                          kind="ExternalInput").ap()
    xik_d = nc.dram_tensor("xik", [128, BC * JR], bf16,
                           kind="ExternalInput").ap()
    wl_d = nc.dram_tensor("wl", [128, NCH * NO], bf16,
                          kind="ExternalInput").ap()
    f_d = nc.dram_tensor("fmat", [128, 128], bf16, kind="ExternalInput").ap()
    y_d = nc.dram_tensor("y", [B_SH, NO], f32, kind="ExternalOutput").ap()

    RG = [list(range(N_CORES))]

    with tile.TileContext(nc) as tc:
        with tc.tile_pool(name="persist", bufs=1) as pp, \
             tc.tile_pool(name="work", bufs=1) as wp, \
             tc.tile_pool(name="ps_s", bufs=2, space="PSUM") as ps_s, \
             tc.tile_pool(name="ps_q", bufs=3, space="PSUM") as ps_q, \
             tc.tile_pool(name="ps_f", bufs=1, space="PSUM") as ps_f, \
             tc.tile_pool(name="dram", bufs=1, space="DRAM") as dp:

            # ---------------- input loads ----------------
            # Replicated full-row tensors for the collective-free iteration 1
            # (fp8: halves the DMA, and s1 only steers routing).  The load +
            # s1 ride the fixed ~33us NEFF-init + CC-boot window, so the
            # first collective (AG of s2) triggers right at the floor.
            xf8_sb = pp.tile([128, FCH, B], f8, name="xf8_sb", tag="xf8_sb")
            wf8_sb = pp.tile([128, FCH, NO], f8, name="wf8_sb", tag="wf8_sb")
            xt_sb = pp.tile([128, NCH, B], bf16, name="xt_sb", tag="xt_sb")
            xik_sb = pp.tile([128, BC, JR], bf16, name="xik_sb", tag="xik_sb")
            wl_sb = pp.tile([128, NCH, NO], bf16, name="wl_sb", tag="wl_sb")
            f_sb = pp.tile([128, 128], bf16, name="f_sb", tag="f_sb")
            b_sb = pp.tile([128, NCH, N_NODE], f32, name="b_sb", tag="b_sb")

            xff = xf8_sb[:].rearrange("p c b -> p (c b)")
            wff = wf8_sb[:].rearrange("p c f -> p (c f)")
            H = FCH // 4
            # interleave xf8 quarters across sync/gpsimd so s1's chunk
            # groups land in consumption order; wf8 on scalar.
            nc.sync.dma_start(xff[:, 0:H * B], xf8_d[:, 0:H * B])
            nc.gpsimd.dma_start(xff[:, H * B:2 * H * B],
                                xf8_d[:, H * B:2 * H * B])
            nc.scalar.dma_start(wff[:, 0:2 * H * NO], wf8_d[:, 0:2 * H * NO])
            nc.sync.dma_start(xff[:, 2 * H * B:3 * H * B],
                              xf8_d[:, 2 * H * B:3 * H * B])
            nc.gpsimd.dma_start(xff[:, 3 * H * B:], xf8_d[:, 3 * H * B:])
            nc.scalar.dma_start(wff[:, 2 * H * NO:], wf8_d[:, 2 * H * NO:])
            # own-shard tensors (iterations 2-3): needed well after s1
            nc.sync.dma_start(
                xik_sb[:].rearrange("p c j -> p (c j)"), xik_d[:])
            nc.scalar.dma_start(f_sb[:], f_d[:])
            nc.scalar.dma_start(
                xt_sb[:].rearrange("p c b -> p (c b)"), xt_d[:])
            nc.scalar.dma_start(
                wl_sb[:].rearrange("p c f -> p (c f)"), wl_d[:])

            wl4 = wl_sb[:].rearrange("p c (n o) -> p c n o", n=N_NODE)

            # ---------------- helpers ----------------
            def s_matmul(rhs3, ar_dsts, dt=f32):
                """ar_dsts[bc] (DRAM) = sum_c xt[:,c,bc].T @ rhs3[:,c,:]
                per batch-chunk: bc0's PSUM->SBUF copy + store DMA overlap
                bc1's matmuls (DMA cannot source PSUM directly)."""
                s_sb = wp.tile([128, BC, NO], dt, name="s_st" + str(dt),
                               tag="s_st" + str(dt))
                for bc_i in range(BC):
                    s_ps = ps_s.tile([128, NO], f32, name="s_ps", tag="s_ps")
                    for c in range(NCH):
                        nc.tensor.matmul(
                            s_ps[:],
                            xt_sb[:, c, bc_i * 128:(bc_i + 1) * 128],
                            rhs3[:, c, :],
                            start=(c == 0), stop=(c == NCH - 1))
                    if bc_i == 0:
                        nc.scalar.copy(s_sb[:, 0, :], s_ps[:])
                        nc.sync.dma_start(ar_dsts[0], s_sb[:, 0, :])
                    else:
                        nc.vector.tensor_copy(s_sb[:, 1, :], s_ps[:])
                        nc.scalar.dma_start(ar_dsts[1], s_sb[:, 1, :])

            def exchange_tiles(t, dt):
                ex_in = dp.tile([128, BC * NO], dt, name=f"ex_in{t}",
                                tag="ex_in")
                if EXCHANGE == "ag8":
                    ex_out = dp.tile([N_CORES * 128, BC * NO], dt,
                                     name=f"ex_out{t}", tag="ex_out",
                                     addr_space="Shared")
                else:
                    ex_out = dp.tile([128, BC * NO], dt, name=f"ex_out{t}",
                                     tag="ex_out", addr_space="Shared")
                return ex_in, ex_out

            def exchange_back(ex_out):
                """AllReduce output -> SBUF.  fp8 comes back through a
                gpsimd cast-DMA (only gpsimd DMAs may cast) as bf16."""
                sf = wp.tile([128, BC, NO], bf16 if EXCHANGE == "arf8"
                             else f32, name="sf", tag="sf")
                nc.gpsimd.dma_start(
                    sf[:].rearrange("p c f -> p (c f)"), ex_out[:])
                return sf

            def tree_reduce(ex_out):
                """fp8 AllGather output [8*128, 320] -> bf16 sum [128, 2, NO].
                Leafs split DVE(3)/GpSimd(1); fp8 reads are the cost."""
                agv = wp.tile([128, N_CORES, BC * NO], f8, name="agv",
                              tag="agv")
                ag3 = ex_out.rearrange("(r p) f -> p r f", p=128)
                nc.sync.dma_start(agv[:, 0:4, :], ag3[:, 0:4, :])
                nc.scalar.dma_start(agv[:, 4:8, :], ag3[:, 4:8, :])
                lf = wp.tile([128, 4, BC * NO], bf16, name="lf", tag="lf")
                for h in range(3):
                    nc.vector.tensor_add(lf[:, h, :], agv[:, 2 * h, :],
                                         agv[:, 2 * h + 1, :])
                nc.gpsimd.tensor_add(lf[:, 3, :], agv[:, 6, :], agv[:, 7, :])
                md = wp.tile([128, 2, BC * NO], bf16, name="md", tag="md")
                nc.vector.tensor_add(md[:], lf[:, 0:2, :], lf[:, 2:4, :])
                sfull = wp.tile([128, BC, NO], f32, name="sfull",
                                tag="sfull")
                nc.vector.tensor_add(
                    sfull[:].rearrange("p c f -> p (c f)"),
                    md[:, 0, :], md[:, 1, :])
                return sfull

            def rsqrt(msq, P, nch, tag, iters):
                """z ~ 1/sqrt(msq) via int bit-trick + Newton steps (DVE
                only -- avoids the Sqrt/Ln ACT table sets entirely)."""
                sh = [P, nch, N_NODE]
                zi = wp.tile(sh, i32, name="zi" + tag, tag="zi" + tag)
                nc.vector.tensor_scalar(
                    out=zi[:], in0=msq[:].bitcast(i32), scalar1=1, scalar2=-1,
                    op0=ALU.arith_shift_right, op1=ALU.bitwise_xor)
                nc.vector.tensor_scalar_add(zi[:], zi[:], RSQRT_MAGIC + 1)
                z = zi[:].bitcast(f32)
                t = wp.tile(sh, f32, name="nt" + tag, tag="nt" + tag)
                w = wp.tile(sh, f32, name="nw" + tag, tag="nw" + tag)
                for _ in range(iters):
                    nc.vector.tensor_mul(t[:], z, z)
                    nc.vector.tensor_mul(t[:], t[:], msq[:])
                    nc.vector.tensor_scalar(
                        out=w[:], in0=t[:], scalar1=-0.5, scalar2=1.5,
                        op0=ALU.mult, op1=ALU.add)
                    nc.vector.tensor_mul(z, z, w[:])
                return z

            def squash(s_ap, P, nch, tag, v_dtype, newton_iters, v_sb=None,
                       v_off=0, scale=None):
                """v = squash(s * scale) over o.  s_ap [P, nch, NO]."""
                s4 = s_ap.rearrange("p c (n o) -> p c n o", n=N_NODE)
                sq = wp.tile([P, nch, NO], f32, name="sq" + tag,
                             tag="sq" + tag)
                nc.vector.tensor_mul(sq[:], s_ap, s_ap)
                msq = wp.tile([P, nch, N_NODE], f32, name="msq" + tag,
                              tag="msq" + tag)
                nc.vector.reduce_sum(
                    msq[:], sq[:].rearrange("p c (n o) -> p c n o", n=N_NODE),
                    axis=AX.X)
                if scale is not None:
                    # s was pre-scale; msq *= scale^2 so fac comes out right,
                    # and the final v-mul absorbs scale via fac*scale.
                    nc.vector.tensor_scalar_mul(msq[:], msq[:],
                                                float(scale * scale))
                den = wp.tile([P, nch, N_NODE], f32, name="den" + tag,
                              tag="den" + tag)
                nc.vector.tensor_scalar_add(den[:], msq[:], 1.0)
                rden = wp.tile([P, nch, N_NODE], f32, name="rden" + tag,
                               tag="rden" + tag)
                nc.vector.reciprocal(rden[:], den[:])
                z = rsqrt(msq, P, nch, tag, newton_iters)
                mag = wp.tile([P, nch, N_NODE], f32, name="mag" + tag,
                              tag="mag" + tag)
                nc.vector.tensor_mul(mag[:], msq[:], z)   # sqrt(msq)
                fac = wp.tile([P, nch, N_NODE], f32, name="fac" + tag,
                              tag="fac" + tag)
                nc.vector.tensor_mul(fac[:], mag[:], rden[:])
                if scale is not None:
                    nc.vector.tensor_scalar_mul(fac[:], fac[:], float(scale))
                if v_sb is None:
                    v_sb = wp.tile([P, nch, NO], v_dtype, name="v" + tag,
                                   tag="v" + tag)
                    v4 = v_sb[:].rearrange("p c (n o) -> p c n o", n=N_NODE)
                else:
                    v4 = v_sb[:, v_off:v_off + nch, :].rearrange(
                        "p c (n o) -> p c n o", n=N_NODE)
                fb = fac[:].unsqueeze(3).broadcast_to((P, nch, N_NODE, O_SZ))
                nc.vector.tensor_mul(v4, s4, fb)
                return v_sb

            def squash_and_q(ex_out, scale=None, direct=None):
                """Squash the exchanged s per batch-chunk, pipelined with the
                Q matmuls; then p = wl*Q (Q staged to bf16 SBUF by the scalar
                engine, split DVE/GpSimd)."""
                if direct is not None:
                    sf = direct
                elif EXCHANGE == "ag8":
                    sf = tree_reduce(ex_out)
                else:
                    sf = exchange_back(ex_out)
                v_sb = wp.tile([128, BC, NO], bf16, name="v_m", tag="v_m")
                q_tiles = []
                for g in range(NCH // 3):
                    q_tiles.append(ps_q.tile([128, 3 * NO], f32, name="q_ps",
                                             tag="q_ps"))
                for bc_i in range(BC):
                    squash(sf[:, bc_i:bc_i + 1, :], 128, 1, "m",
                           bf16, NEWTON_ROUTE, v_sb=v_sb, v_off=bc_i,
                           scale=scale)
                    for g in range(NCH // 3):
                        for s_i in range(3):
                            mc = g * 3 + s_i
                            nc.tensor.matmul(
                                q_tiles[g][:, s_i * NO:(s_i + 1) * NO],
                                xik_sb[:, bc_i, mc * 128:(mc + 1) * 128],
                                v_sb[:, bc_i, :],
                                start=(bc_i == 0), stop=(bc_i == BC - 1))
                q_sb = wp.tile([128, NCH, NO], bf16, name="q_sb", tag="q_sb")
                p_sb = wp.tile([128, NCH, NO], bf16, name="p_sb", tag="p_sb")
                pr = wp.tile([128, NCH, N_NODE], f32, name="pr_sb",
                             tag="pr_sb")
                for g in range(NCH // 3):
                    gs = slice(g * 3, (g + 1) * 3)
                    g2 = slice(g * 3, g * 3 + 2)
                    nc.scalar.copy(
                        q_sb[:, gs, :].rearrange("p c f -> p (c f)"),
                        q_tiles[g][:])
                    nc.vector.tensor_mul(p_sb[:, g2, :], wl_sb[:, g2, :],
                                         q_sb[:, g2, :])
                    nc.gpsimd.tensor_mul(p_sb[:, g * 3 + 2, :],
                                         wl_sb[:, g * 3 + 2, :],
                                         q_sb[:, g * 3 + 2, :])
                    nc.vector.reduce_sum(
                        pr[:, gs, :],
                        p_sb[:, gs, :].rearrange(
                            "p c (n o) -> p c n o", n=N_NODE),
                        axis=AX.X)
                return v_sb, pr

            def b_update(pr, first):
                prb = wp.tile([128, NCH, N_NODE], bf16, name="prb", tag="prb")
                nc.vector.tensor_copy(prb[:], pr[:])
                uv_ps = ps_f.tile([128, NCH * N_NODE], f32, name="uv_ps",
                                  tag="uv_ps")
                nc.tensor.matmul(uv_ps[:], f_sb[:],
                                 prb[:].rearrange("p c n -> p (c n)"),
                                 start=True, stop=True)
                uv3 = uv_ps[:].rearrange("p (c n) -> p c n", n=N_NODE)
                if first:
                    # keep b state for the next update, but let the softmax
                    # read the PSUM uv directly (shorter critical path)
                    nc.scalar.copy(b_sb[:], uv3)
                    return uv3
                nc.vector.tensor_add(b_sb[:], b_sb[:], uv3)
                return b_sb[:]

            def softmax_mc(b_src):
                e_sb = wp.tile([128, NCH, N_NODE], f32, name="e_sb",
                               tag="e_sb")
                nc.scalar.activation(e_sb[:], b_src, AF.Exp)
                se = wp.tile([128, NCH], f32, name="se", tag="se")
                nc.vector.reduce_sum(se[:], e_sb[:], axis=AX.X)
                rse = wp.tile([128, NCH], f32, name="rse", tag="rse")
                nc.vector.reciprocal(rse[:], se[:])
                c_sb = wp.tile([128, NCH, N_NODE], bf16, name="c_sb",
                               tag="c_sb")
                nc.vector.tensor_mul(
                    c_sb[:], e_sb[:],
                    rse[:].unsqueeze(2).broadcast_to((128, NCH, N_NODE)))
                mc_sb = wp.tile([128, NCH, NO], bf16, name="mc_sb",
                                tag="mc_sb")
                cb = c_sb[:].unsqueeze(3).broadcast_to(
                    (128, NCH, N_NODE, O_SZ))
                mc4 = mc_sb[:].rearrange("p c (n o) -> p c n o", n=N_NODE)
                # split the W-sized multiply across DVE and the idle GpSimd
                nc.vector.tensor_mul(mc4[:, 0:8], wl4[:, 0:8], cb[:, 0:8])
                nc.gpsimd.tensor_mul(mc4[:, 8:NCH], wl4[:, 8:NCH],
                                     cb[:, 8:NCH])
                return mc_sb

            ex_dt = f32 if EXCHANGE == "arf32" else f8
            ex_kind = "AllGather" if EXCHANGE == "ag8" else "AllReduce"
            ex_op = ALU.bypass if EXCHANGE == "ag8" else ALU.add

            # -------- iteration 1: replicated full s1, no collective ------
            # s1_psum = sum_j xf8.T @ wf8 over ALL 9216 rows; wf8 = 8*Wl on
            # the host (dodges fp8 subnormals), so true s1 = (0.1/8)*s1_psum
            # -- the 0.0125 is folded into the squash.
            sf1 = wp.tile([128, BC, NO], f32, name="sf1", tag="sf1")
            for bc_i in range(BC):
                s_ps = ps_s.tile([128, NO], f32, name="s_ps", tag="s_ps")
                for c in range(FCH):
                    nc.tensor.matmul(
                        s_ps[:],
                        xf8_sb[:, c, bc_i * 128:(bc_i + 1) * 128],
                        wf8_sb[:, c, :],
                        start=(c == 0), stop=(c == FCH - 1))
                if bc_i == 0:
                    nc.scalar.copy(sf1[:, 0, :], s_ps[:])
                else:
                    nc.vector.tensor_copy(sf1[:, 1, :], s_ps[:])
            v_sb, pr = squash_and_q(None, scale=0.1 / 8.0, direct=sf1[:])
            b_src = b_update(pr, first=True)

            # ------------- iteration 2 (first collective) -----------------
            mc_sb = softmax_mc(b_src)
            ex_in, ex_out = exchange_tiles(0, ex_dt)
            s_matmul(mc_sb[:], [ex_in[:, 0:NO], ex_in[:, NO:2 * NO]],
                     dt=ex_dt)
            nc.gpsimd.collective_compute(
                ex_kind, ex_op, replica_groups=RG,
                ins=[ex_in.opt()], outs=[ex_out.opt()])
            v_sb, pr = squash_and_q(ex_out)
            b_src = b_update(pr, first=False)

            # ---------------- iteration 3 (no b-update) ----------------
            mc_sb = softmax_mc(b_src)
            rs_in = dp.tile([B, NO], f32, name="rs_in", tag="rs_in")
            rs_out = dp.tile([B_SH, NO], f32, name="rs_out", tag="rs_out")
            rs2 = rs_in.rearrange("(c p) f -> p c f", p=128)
            s_matmul(mc_sb[:], [rs2[:, 0, :], rs2[:, 1, :]])
            nc.gpsimd.collective_compute(
                "ReduceScatter", ALU.add, replica_groups=RG,
                ins=[rs_in.opt()], outs=[rs_out.opt()])
            ssh = wp.tile([B_SH, 1, NO], f32, name="ssh", tag="ssh")
            nc.sync.dma_start(ssh[:, 0, :], rs_out[:])
            vsh = squash(ssh[:], B_SH, 1, "s", f32, 1)
            nc.sync.dma_start(y_d[:], vsh[:, 0, :])

    nc.compile()
    return nc


def _host_prep(x, W):
    """Per-core input dicts (partition-major layouts) + the F matrix."""
    import ml_dtypes

    bf = ml_dtypes.bfloat16
    x = np.ascontiguousarray(x, dtype=np.float32)
    W = np.ascontiguousarray(W, dtype=np.float32)
    F = (np.kron(np.eye(16, dtype=np.float32),
                 np.ones((8, 8), dtype=np.float32)) / np.float32(B)).astype(bf)
    f8 = ml_dtypes.float8_e4m3
    # replicated full-row tensors, shard-major row order j=(core,i_loc,k)
    xt_full = np.ascontiguousarray(x.transpose(2, 1, 0)).reshape(
        I_TOT * IN_SIZE, B)
    xf8 = np.ascontiguousarray(
        xt_full.reshape(FCH, 128, B).transpose(1, 0, 2)).reshape(
            128, FCH * B).astype(f8)
    wl_full = np.ascontiguousarray(
        (np.float32(0.24) * W[0]).transpose(0, 3, 1, 2)).reshape(
            I_TOT * IN_SIZE, NO)
    wf8 = np.ascontiguousarray(
        wl_full.reshape(FCH, 128, NO).transpose(1, 0, 2)).reshape(
            128, FCH * NO).astype(f8)
    in_maps = []
    for c in range(N_CORES):
        sl = slice(c * I_SH, (c + 1) * I_SH)
        x_sh = x[:, :, sl]                                   # [B, K, I_SH]
        # xt rows j=(i,k): [JR, B] -> partition-major [128, NCH, B]
        xt = np.ascontiguousarray(x_sh.transpose(2, 1, 0)).reshape(JR, B)
        xt_pm = np.ascontiguousarray(
            xt.reshape(NCH, 128, B).transpose(1, 0, 2)).reshape(128, NCH * B)
        # xik [B, JR] -> [128, BC, JR]
        xik = np.ascontiguousarray(
            x_sh.transpose(0, 2, 1)).reshape(B, JR)
        xik_pm = np.ascontiguousarray(
            xik.reshape(BC, 128, JR).transpose(1, 0, 2)).reshape(
                128, BC * JR)
        # wl rows j: [JR, NO] -> [128, NCH, NO]
        wlf = np.ascontiguousarray(
            (np.float32(0.03) * W[0, sl]).transpose(0, 3, 1, 2)
        ).reshape(JR, NO)
        wl_pm = np.ascontiguousarray(
            wlf.reshape(NCH, 128, NO).transpose(1, 0, 2)).reshape(
                128, NCH * NO)
        m = {"xf8": xf8, "wf8": wf8, "xt": xt_pm.astype(bf),
             "xik": xik_pm.astype(bf), "wl": wl_pm.astype(bf), "fmat": F}
        in_maps.append(m)
    return in_maps


def _run(in_maps, trace=False, all_cores=False):
    from concourse.bass_utils import run_bass_kernel_spmd

    if "nc" not in _CACHE:
        _CACHE["nc"] = _build_program()
    nc = _CACHE["nc"]
    kwargs = {}
    if all_cores:
        kwargs["trace_cores"] = list(range(N_CORES))
    res = run_bass_kernel_spmd(nc, in_maps, core_ids=list(range(N_CORES)),
                               trace=trace, **kwargs)
    return res


def kernel(x: np.ndarray, W: np.ndarray) -> np.ndarray:
    in_maps = _host_prep(x, W)
    res = _run(in_maps)
    v = np.concatenate([res.results[c]["y"] for c in range(N_CORES)], axis=0)
    return v.reshape(B, N_NODE, O_SZ, 1).astype(np.float32)


# revision 19
# speedup vs baseline: 2.1049x; 2.1049x over previous
"""CapsuleLayer (dynamic routing, 3 iterations) on 8 Trainium2 NeuronCores.

Decomposition (never materializes u_hat = [256,1152,10,16], 189MB):
  - Shard the 1152 input capsules (i) 8 ways: 144 per core.
  - Per-core row space j = (i_local, k), k = in_size = 8 -> 1152 rows
    = 9 chunks of 128 partitions.
  - s_j:  s[b,(n,o)] = sum_j xT[j,b] * (c[j,n] * Wl[j,(n,o)])   (PE matmul,
    contraction over j; Wl = 0.03*W in [(i,k),(n,o)] layout, c broadcast
    over k and o).  Partial over the i-shard -> summed across cores by a
    fp32 AllReduce straight out of PSUM (no SBUF copies, no on-chip tree).
  - b_ij update via a Gram matrix instead of u_hat:
       Q[j,(n,o)]  = sum_b x[b,j] * v[b,(n,o)]                  (PE matmul)
       pr[j,n]     = sum_o Wl[j,(n,o)] * Q[j,(n,o)]             (DVE)
       uv_rows     = F.T @ pr  per 128-chunk, F = kron(I16, ones8x8)/B
                     (sums over k within each i-group AND replicates the
                     result back to all k-rows, so b stays row-replicated)
  - Iteration 1 uses uniform c = 1/10 (softmax of zeros): s1 = 0.1*(xT.T@Wl).
  - Iteration 3 needs no b-update; the fp32 s3 goes through ReduceScatter
    (also straight out of PSUM) so each core squashes only its 32-row
    batch shard; the host just concatenates the 8 shards.

Latency plan (the kernel is serial-latency-bound, engines are <20% busy):
  - The 8-core rendezvous barrier releases when the LAST core triggers its
    first collective, so per-core time-to-first-trigger is on every core's
    critical path.  All inputs are host-transposed to partition-major so
    each loads with ONE plain 2D contiguous DMA (3D-strided dma_starts
    cost ~1.3-1.8us of descriptor generation each; 2D cost ~0.6us).
  - Routing matmuls use bf16 operands (fp32 PE matmuls are 4x slower).
  - squash per 128-batch-chunk is pipelined with the Q matmuls of the
    previous chunk; the W-sized mc = c*Wl multiply is split DVE/GpSimd.
  - sqrt is a bit-trick + Newton on the DVE so the ScalarE only ever needs
    the Exp activation table (Sqrt/Ln live in other table sets and would
    force ~2.7us ACT_TABLE_LOADs per iteration).  The routing squashes
    skip the Newton step entirely (~3.4% rsqrt error, which averages out
    across the 1152-capsule contraction); the output squash uses two.
"""
import sys

if "/opt/trn_rl_repo" not in sys.path:
    sys.path.insert(0, "/opt/trn_rl_repo")

import numpy as np

import os
N_CORES = int(os.environ.get("KERNEL_CORES", "8"))
B, IN_SIZE, I_TOT = 256, 8, 1152
N_NODE, O_SZ = 10, 16
NO = N_NODE * O_SZ          # 160
I_SH = I_TOT // N_CORES     # 144 capsules per core
JR = I_SH * IN_SIZE         # 1152 rows per core
NCH = JR // 128             # 9 contraction chunks
BC = B // 128               # 2 batch chunks
FCH = I_TOT * IN_SIZE // 128  # 72 full-row chunks (replicated iteration 1)
B_SH = B // N_CORES         # 32 batch rows per core after ReduceScatter

RSQRT_MAGIC = 0x5F3759DF
NEWTON_ROUTE = int(os.environ.get("KERNEL_NEWTON_ROUTE", "0"))
EXCHANGE = os.environ.get("KERNEL_EXCHANGE", "ag8")

_CACHE = {}


def _build_program():
    import concourse.bacc as bacc
    import concourse.tile as tile
    import concourse.mybir as mybir

    f32 = mybir.dt.float32
    bf16 = mybir.dt.bfloat16
    f8 = mybir.dt.float8e4
    i32 = mybir.dt.int32
    AF = mybir.ActivationFunctionType
    ALU = mybir.AluOpType
    AX = mybir.AxisListType

    nc = bacc.Bacc("TRN2", target_bir_lowering=False, debug=False,
                   enable_asserts=False, num_devices=N_CORES)

    # All inputs partition-major: one plain 2D contiguous DMA each.
    xf8_d = nc.dram_tensor("xf8", [128, FCH * B], f8,
                           kind="ExternalInput").ap()
    wf8_d = nc.dram_tensor("wf8", [128, FCH * NO], f8,
                           kind="ExternalInput").ap()
    xt_d = nc.dram_tensor("xt", [128, NCH * B], bf16,
                          kind="ExternalInput").ap()
    xik_d = nc.dram_tensor("xik", [128, BC * JR], bf16,
                           kind="ExternalInput").ap()
    wl_d = nc.dram_tensor("wl", [128, NCH * NO], bf16,
                          kind="ExternalInput").ap()
    f_d = nc.dram_tensor("fmat", [128, 128], bf16, kind="ExternalInput").ap()
    y_d = nc.dram_tensor("y", [B_SH, NO], f32, kind="ExternalOutput").ap()

    RG = [list(range(N_CORES))]

    with tile.TileContext(nc) as tc:
        with tc.tile_pool(name="persist", bufs=1) as pp, \
             tc.tile_pool(name="work", bufs=1) as wp, \
             tc.tile_pool(name="ps_s", bufs=2, space="PSUM") as ps_s, \
             tc.tile_pool(name="ps_q", bufs=3, space="PSUM") as ps_q, \
             tc.tile_pool(name="ps_f", bufs=1, space="PSUM") as ps_f, \
             tc.tile_pool(name="dram", bufs=1, space="DRAM") as dp:

            # ---------------- input loads ----------------
            # Replicated full-row tensors for the collective-free iteration 1
            # (fp8: halves the DMA, and s1 only steers routing).  The load +
            # s1 ride the fixed ~33us NEFF-init + CC-boot window, so the
            # first collective (AG of s2) triggers right at the floor.
            xf8_sb = pp.tile([128, FCH, B], f8, name="xf8_sb", tag="xf8_sb")
            wf8_sb = pp.tile([128, FCH, NO], f8, name="wf8_sb", tag="wf8_sb")
            xt_sb = pp.tile([128, NCH, B], bf16, name="xt_sb", tag="xt_sb")
            xik_sb = pp.tile([128, BC, JR], bf16, name="xik_sb", tag="xik_sb")
            wl_sb = pp.tile([128, NCH, NO], bf16, name="wl_sb", tag="wl_sb")
            f_sb = pp.tile([128, 128], bf16, name="f_sb", tag="f_sb")
            b_sb = pp.tile([128, NCH, N_NODE], f32, name="b_sb", tag="b_sb")

            xff = xf8_sb[:].rearrange("p c b -> p (c b)")
            wff = wf8_sb[:].rearrange("p c f -> p (c f)")
            H = FCH // 4
            # interleave xf8 quarters across sync/gpsimd so s1's chunk
            # groups land in consumption order; wf8 on scalar.
            nc.sync.dma_start(xff[:, 0:H * B], xf8_d[:, 0:H * B])
            nc.gpsimd.dma_start(xff[:, H * B:2 * H * B],
                                xf8_d[:, H * B:2 * H * B])
            nc.scalar.dma_start(wff[:, 0:2 * H * NO], wf8_d[:, 0:2 * H * NO])
            nc.sync.dma_start(xff[:, 2 * H * B:3 * H * B],
                              xf8_d[:, 2 * H * B:3 * H * B])
            nc.gpsimd.dma_start(xff[:, 3 * H * B:], xf8_d[:, 3 * H * B:])
            nc.scalar.dma_start(wff[:, 2 * H * NO:], wf8_d[:, 2 * H * NO:])
            # own-shard tensors (iterations 2-3): needed well after s1
            nc.sync.dma_start(
                xik_sb[:].rearrange("p c j -> p (c j)"), xik_d[:])
            nc.scalar.dma_start(f_sb[:], f_d[:])
            nc.scalar.dma_start(
                xt_sb[:].rearrange("p c b -> p (c b)"), xt_d[:])
            nc.scalar.dma_start(
                wl_sb[:].rearrange("p c f -> p (c f)"), wl_d[:])

            wl4 = wl_sb[:].rearrange("p c (n o) -> p c n o", n=N_NODE)

            # ---------------- helpers ----------------
            def s_matmul(rhs3, ar_dsts, dt=f32):
                """ar_dsts[bc] (DRAM) = sum_c xt[:,c,bc].T @ rhs3[:,c,:]
                per batch-chunk: bc0's PSUM->SBUF copy + store DMA overlap
                bc1's matmuls (DMA cannot source PSUM directly)."""
                s_sb = wp.tile([128, BC, NO], dt, name="s_st" + str(dt),
                               tag="s_st" + str(dt))
                for bc_i in range(BC):
                    s_ps = ps_s.tile([128, NO], f32, name="s_ps", tag="s_ps")
                    for c in range(NCH):
                        nc.tensor.matmul(
                            s_ps[:],
                            xt_sb[:, c, bc_i * 128:(bc_i + 1) * 128],
                            rhs3[:, c, :],
                            start=(c == 0), stop=(c == NCH - 1))
                    if bc_i == 0:
                        nc.scalar.copy(s_sb[:, 0, :], s_ps[:])
                        nc.sync.dma_start(ar_dsts[0], s_sb[:, 0, :])
                    else:
                        nc.vector.tensor_copy(s_sb[:, 1, :], s_ps[:])
                        nc.scalar.dma_start(ar_dsts[1], s_sb[:, 1, :])

            def exchange_tiles(t, dt):
                ex_in = dp.tile([128, BC * NO], dt, name=f"ex_in{t}",
                                tag="ex_in")
                if EXCHANGE == "ag8":
                    ex_out = dp.tile([N_CORES * 128, BC * NO], dt,
                                     name=f"ex_out{t}", tag="ex_out",
                                     addr_space="Shared")
                else:
                    ex_out = dp.tile([128, BC * NO], dt, name=f"ex_out{t}",
                                     tag="ex_out", addr_space="Shared")
                return ex_in, ex_out

            def exchange_back(ex_out):
                """AllReduce output -> SBUF.  fp8 comes back through a
                gpsimd cast-DMA (only gpsimd DMAs may cast) as bf16."""
                sf = wp.tile([128, BC, NO], bf16 if EXCHANGE == "arf8"
                             else f32, name="sf", tag="sf")
                nc.gpsimd.dma_start(
                    sf[:].rearrange("p c f -> p (c f)"), ex_out[:])
                return sf

            def tree_half(exo, sfull, bc_i):
                """One half [8*128, NO] fp8 -> sfull[:, bc_i, :] bf16 sum."""
                agv = wp.tile([128, N_CORES, NO], f8, name=f"agv{bc_i}",
                              tag=f"agv{bc_i}")
                ag3 = exo.rearrange("(r p) f -> p r f", p=128)
                nc.sync.dma_start(agv[:, 0:4, :], ag3[:, 0:4, :])
                nc.scalar.dma_start(agv[:, 4:8, :], ag3[:, 4:8, :])
                lf = wp.tile([128, 4, NO], bf16, name=f"lf{bc_i}",
                             tag=f"lf{bc_i}")
                for h in range(3):
                    nc.vector.tensor_add(lf[:, h, :], agv[:, 2 * h, :],
                                         agv[:, 2 * h + 1, :])
                nc.gpsimd.tensor_add(lf[:, 3, :], agv[:, 6, :], agv[:, 7, :])
                md = wp.tile([128, 2, NO], bf16, name=f"md{bc_i}",
                             tag=f"md{bc_i}")
                nc.vector.tensor_add(md[:], lf[:, 0:2, :], lf[:, 2:4, :])
                nc.vector.tensor_add(sfull[:, bc_i, :], md[:, 0, :],
                                     md[:, 1, :])

            def rsqrt(msq, P, nch, tag, iters):
                """z ~ 1/sqrt(msq) via int bit-trick + Newton steps (DVE
                only -- avoids the Sqrt/Ln ACT table sets entirely)."""
                sh = [P, nch, N_NODE]
                zi = wp.tile(sh, i32, name="zi" + tag, tag="zi" + tag)
                nc.vector.tensor_scalar(
                    out=zi[:], in0=msq[:].bitcast(i32), scalar1=1, scalar2=-1,
                    op0=ALU.arith_shift_right, op1=ALU.bitwise_xor)
                nc.vector.tensor_scalar_add(zi[:], zi[:], RSQRT_MAGIC + 1)
                z = zi[:].bitcast(f32)
                t = wp.tile(sh, f32, name="nt" + tag, tag="nt" + tag)
                w = wp.tile(sh, f32, name="nw" + tag, tag="nw" + tag)
                for _ in range(iters):
                    nc.vector.tensor_mul(t[:], z, z)
                    nc.vector.tensor_mul(t[:], t[:], msq[:])
                    nc.vector.tensor_scalar(
                        out=w[:], in0=t[:], scalar1=-0.5, scalar2=1.5,
                        op0=ALU.mult, op1=ALU.add)
                    nc.vector.tensor_mul(z, z, w[:])
                return z

            def squash(s_ap, P, nch, tag, v_dtype, newton_iters, v_sb=None,
                       v_off=0, scale=None):
                """v = squash(s * scale) over o.  s_ap [P, nch, NO]."""
                s4 = s_ap.rearrange("p c (n o) -> p c n o", n=N_NODE)
                sq = wp.tile([P, nch, NO], f32, name="sq" + tag,
                             tag="sq" + tag)
                nc.vector.tensor_mul(sq[:], s_ap, s_ap)
                msq = wp.tile([P, nch, N_NODE], f32, name="msq" + tag,
                              tag="msq" + tag)
                nc.vector.reduce_sum(
                    msq[:], sq[:].rearrange("p c (n o) -> p c n o", n=N_NODE),
                    axis=AX.X)
                if scale is not None:
                    # s was pre-scale; msq *= scale^2 so fac comes out right,
                    # and the final v-mul absorbs scale via fac*scale.
                    nc.vector.tensor_scalar_mul(msq[:], msq[:],
                                                float(scale * scale))
                den = wp.tile([P, nch, N_NODE], f32, name="den" + tag,
                              tag="den" + tag)
                nc.vector.tensor_scalar_add(den[:], msq[:], 1.0)
                rden = wp.tile([P, nch, N_NODE], f32, name="rden" + tag,
                               tag="rden" + tag)
                nc.vector.reciprocal(rden[:], den[:])
                z = rsqrt(msq, P, nch, tag, newton_iters)
                mag = wp.tile([P, nch, N_NODE], f32, name="mag" + tag,
                              tag="mag" + tag)
                nc.vector.tensor_mul(mag[:], msq[:], z)   # sqrt(msq)
                fac = wp.tile([P, nch, N_NODE], f32, name="fac" + tag,
                              tag="fac" + tag)
                nc.vector.tensor_mul(fac[:], mag[:], rden[:])
                if scale is not None:
                    nc.vector.tensor_scalar_mul(fac[:], fac[:], float(scale))
                if v_sb is None:
                    v_sb = wp.tile([P, nch, NO], v_dtype, name="v" + tag,
                                   tag="v" + tag)
                    v4 = v_sb[:].rearrange("p c (n o) -> p c n o", n=N_NODE)
                else:
                    v4 = v_sb[:, v_off:v_off + nch, :].rearrange(
                        "p c (n o) -> p c n o", n=N_NODE)
                fb = fac[:].unsqueeze(3).broadcast_to((P, nch, N_NODE, O_SZ))
                nc.vector.tensor_mul(v4, s4, fb)
                return v_sb

            def squash_and_q(ex_out, scale=None, direct=None):
                """Squash the exchanged s per batch-chunk, pipelined with the
                Q matmuls; then p = wl*Q (Q staged to bf16 SBUF by the scalar
                engine, split DVE/GpSimd)."""
                halves = None
                if direct is not None:
                    sf = direct
                elif EXCHANGE == "ag8":
                    halves = ex_out
                    sf = wp.tile([128, BC, NO], f32, name="sfull",
                                 tag="sfull")
                else:
                    sf = exchange_back(ex_out)
                v_sb = wp.tile([128, BC, NO], bf16, name="v_m", tag="v_m")
                q_tiles = []
                for g in range(NCH // 3):
                    q_tiles.append(ps_q.tile([128, 3 * NO], f32, name="q_ps",
                                             tag="q_ps"))
                for bc_i in range(BC):
                    if halves is not None:
                        tree_half(halves[bc_i], sf, bc_i)
                    squash(sf[:, bc_i:bc_i + 1, :], 128, 1, "m",
                           bf16, NEWTON_ROUTE, v_sb=v_sb, v_off=bc_i,
                           scale=scale)
                    for g in range(NCH // 3):
                        for s_i in range(3):
                            mc = g * 3 + s_i
                            nc.tensor.matmul(
                                q_tiles[g][:, s_i * NO:(s_i + 1) * NO],
                                xik_sb[:, bc_i, mc * 128:(mc + 1) * 128],
                                v_sb[:, bc_i, :],
                                start=(bc_i == 0), stop=(bc_i == BC - 1))
                q_sb = wp.tile([128, NCH, NO], bf16, name="q_sb", tag="q_sb")
                p_sb = wp.tile([128, NCH, NO], bf16, name="p_sb", tag="p_sb")
                pr = wp.tile([128, NCH, N_NODE], f32, name="pr_sb",
                             tag="pr_sb")
                for g in range(NCH // 3):
                    gs = slice(g * 3, (g + 1) * 3)
                    g2 = slice(g * 3, g * 3 + 2)
                    nc.scalar.copy(
                        q_sb[:, gs, :].rearrange("p c f -> p (c f)"),
                        q_tiles[g][:])
                    nc.vector.tensor_mul(p_sb[:, g2, :], wl_sb[:, g2, :],
                                         q_sb[:, g2, :])
                    nc.gpsimd.tensor_mul(p_sb[:, g * 3 + 2, :],
                                         wl_sb[:, g * 3 + 2, :],
                                         q_sb[:, g * 3 + 2, :])
                    nc.vector.reduce_sum(
                        pr[:, gs, :],
                        p_sb[:, gs, :].rearrange(
                            "p c (n o) -> p c n o", n=N_NODE),
                        axis=AX.X)
                return v_sb, pr

            def b_update(pr, first):
                prb = wp.tile([128, NCH, N_NODE], bf16, name="prb", tag="prb")
                nc.vector.tensor_copy(prb[:], pr[:])
                uv_ps = ps_f.tile([128, NCH * N_NODE], f32, name="uv_ps",
                                  tag="uv_ps")
                nc.tensor.matmul(uv_ps[:], f_sb[:],
                                 prb[:].rearrange("p c n -> p (c n)"),
                                 start=True, stop=True)
                uv3 = uv_ps[:].rearrange("p (c n) -> p c n", n=N_NODE)
                if first:
                    # keep b state for the next update, but let the softmax
                    # read the PSUM uv directly (shorter critical path)
                    nc.scalar.copy(b_sb[:], uv3)
                    return uv3
                nc.vector.tensor_add(b_sb[:], b_sb[:], uv3)
                return b_sb[:]

            def softmax_mc(b_src):
                e_sb = wp.tile([128, NCH, N_NODE], f32, name="e_sb",
                               tag="e_sb")
                nc.scalar.activation(e_sb[:], b_src, AF.Exp)
                se = wp.tile([128, NCH], f32, name="se", tag="se")
                nc.vector.reduce_sum(se[:], e_sb[:], axis=AX.X)
                rse = wp.tile([128, NCH], f32, name="rse", tag="rse")
                nc.vector.reciprocal(rse[:], se[:])
                c_sb = wp.tile([128, NCH, N_NODE], bf16, name="c_sb",
                               tag="c_sb")
                nc.vector.tensor_mul(
                    c_sb[:], e_sb[:],
                    rse[:].unsqueeze(2).broadcast_to((128, NCH, N_NODE)))
                mc_sb = wp.tile([128, NCH, NO], bf16, name="mc_sb",
                                tag="mc_sb")
                cb = c_sb[:].unsqueeze(3).broadcast_to(
                    (128, NCH, N_NODE, O_SZ))
                mc4 = mc_sb[:].rearrange("p c (n o) -> p c n o", n=N_NODE)
                # split across DVE (fine-grained, so the s-matmuls start
                # after the first sub-op) and the idle GpSimd
                nc.vector.tensor_mul(mc4[:, 0:3], wl4[:, 0:3], cb[:, 0:3])
                nc.vector.tensor_mul(mc4[:, 3:6], wl4[:, 3:6], cb[:, 3:6])
                nc.vector.tensor_mul(mc4[:, 6:8], wl4[:, 6:8], cb[:, 6:8])
                nc.gpsimd.tensor_mul(mc4[:, 8:NCH], wl4[:, 8:NCH],
                                     cb[:, 8:NCH])
                return mc_sb

            ex_dt = f32 if EXCHANGE == "arf32" else f8
            ex_kind = "AllGather" if EXCHANGE == "ag8" else "AllReduce"
            ex_op = ALU.bypass if EXCHANGE == "ag8" else ALU.add

            # -------- iteration 1: replicated full s1, no collective ------
            # s1_psum = sum_j xf8.T @ wf8 over ALL 9216 rows; wf8 = 8*Wl on
            # the host (dodges fp8 subnormals), so true s1 = (0.1/8)*s1_psum
            # -- the 0.0125 is folded into the squash.
            sf1 = wp.tile([128, BC, NO], f32, name="sf1", tag="sf1")
            for bc_i in range(BC):
                s_ps = ps_s.tile([128, NO], f32, name="s_ps", tag="s_ps")
                for c in range(FCH):
                    nc.tensor.matmul(
                        s_ps[:],
                        xf8_sb[:, c, bc_i * 128:(bc_i + 1) * 128],
                        wf8_sb[:, c, :],
                        start=(c == 0), stop=(c == FCH - 1))
                if bc_i == 0:
                    nc.scalar.copy(sf1[:, 0, :], s_ps[:])
                else:
                    nc.vector.tensor_copy(sf1[:, 1, :], s_ps[:])
            v_sb, pr = squash_and_q(None, scale=0.1 / 8.0, direct=sf1[:])
            b_src = b_update(pr, first=True)

            # ------------- iteration 2 (first collectives) ----------------
            # The s2 exchange is split per 128-batch-chunk into two AGs on
            # the CC stream: AG-b runs while the core tree-reduces and
            # squashes half a, hiding most of its latency.
            mc_sb = softmax_mc(b_src)
            ex_a, ex_b, exo_a, exo_b = None, None, None, None
            if EXCHANGE == "ag8":
                ex_a = dp.tile([128, NO], f8, name="exa", tag="exa")
                ex_b = dp.tile([128, NO], f8, name="exb", tag="exb")
                exo_a = dp.tile([N_CORES * 128, NO], f8, name="exoa",
                                tag="exoa", addr_space="Shared")
                exo_b = dp.tile([N_CORES * 128, NO], f8, name="exob",
                                tag="exob", addr_space="Shared")
                s_matmul(mc_sb[:], [ex_a[:], ex_b[:]], dt=f8)
                nc.gpsimd.collective_compute(
                    "AllGather", ALU.bypass, replica_groups=RG,
                    ins=[ex_a.opt()], outs=[exo_a.opt()])
                nc.gpsimd.collective_compute(
                    "AllGather", ALU.bypass, replica_groups=RG,
                    ins=[ex_b.opt()], outs=[exo_b.opt()])
                v_sb, pr = squash_and_q((exo_a, exo_b))
            else:
                ex_in, ex_out = exchange_tiles(0, ex_dt)
                s_matmul(mc_sb[:], [ex_in[:, 0:NO], ex_in[:, NO:2 * NO]],
                         dt=ex_dt)
                nc.gpsimd.collective_compute(
                    ex_kind, ex_op, replica_groups=RG,
                    ins=[ex_in.opt()], outs=[ex_out.opt()])
                v_sb, pr = squash_and_q(ex_out)
            b_src = b_update(pr, first=False)

            # ---------------- iteration 3 (no b-update) ----------------
            mc_sb = softmax_mc(b_src)
            rs_in = dp.tile([B, NO], f32, name="rs_in", tag="rs_in")
            rs_out = dp.tile([B_SH, NO], f32, name="rs_out", tag="rs_out")
            rs2 = rs_in.rearrange("(c p) f -> p c f", p=128)
            s_matmul(mc_sb[:], [rs2[:, 0, :], rs2[:, 1, :]])
            nc.gpsimd.collective_compute(
                "ReduceScatter", ALU.add, replica_groups=RG,
                ins=[rs_in.opt()], outs=[rs_out.opt()])
            ssh = wp.tile([B_SH, 1, NO], f32, name="ssh", tag="ssh")
            nc.sync.dma_start(ssh[:, 0, :], rs_out[:])
            vsh = squash(ssh[:], B_SH, 1, "s", f32, 1)
            nc.sync.dma_start(y_d[:], vsh[:, 0, :])

    nc.compile()
    return nc


def _host_prep(x, W):
    """Per-core input dicts (partition-major layouts) + the F matrix."""
    import ml_dtypes

    bf = ml_dtypes.bfloat16
    x = np.ascontiguousarray(x, dtype=np.float32)
    W = np.ascontiguousarray(W, dtype=np.float32)
    F = (np.kron(np.eye(16, dtype=np.float32),
                 np.ones((8, 8), dtype=np.float32)) / np.float32(B)).astype(bf)
    f8 = ml_dtypes.float8_e4m3
    # replicated full-row tensors, shard-major row order j=(core,i_loc,k)
    xt_full = np.ascontiguousarray(x.transpose(2, 1, 0)).reshape(
        I_TOT * IN_SIZE, B)
    xf8 = np.ascontiguousarray(
        xt_full.reshape(FCH, 128, B).transpose(1, 0, 2)).reshape(
            128, FCH * B).astype(f8)
    wl_full = np.ascontiguousarray(
        (np.float32(0.24) * W[0]).transpose(0, 3, 1, 2)).reshape(
            I_TOT * IN_SIZE, NO)
    wf8 = np.ascontiguousarray(
        wl_full.reshape(FCH, 128, NO).transpose(1, 0, 2)).reshape(
            128, FCH * NO).astype(f8)
    in_maps = []
    for c in range(N_CORES):
        sl = slice(c * I_SH, (c + 1) * I_SH)
        x_sh = x[:, :, sl]                                   # [B, K, I_SH]
        # xt rows j=(i,k): [JR, B] -> partition-major [128, NCH, B]
        xt = np.ascontiguousarray(x_sh.transpose(2, 1, 0)).reshape(JR, B)
        xt_pm = np.ascontiguousarray(
            xt.reshape(NCH, 128, B).transpose(1, 0, 2)).reshape(128, NCH * B)
        # xik [B, JR] -> [128, BC, JR]
        xik = np.ascontiguousarray(
            x_sh.transpose(0, 2, 1)).reshape(B, JR)
        xik_pm = np.ascontiguousarray(
            xik.reshape(BC, 128, JR).transpose(1, 0, 2)).reshape(
                128, BC * JR)
        # wl rows j: [JR, NO] -> [128, NCH, NO]
        wlf = np.ascontiguousarray(
            (np.float32(0.03) * W[0, sl]).transpose(0, 3, 1, 2)
        ).reshape(JR, NO)
        wl_pm = np.ascontiguousarray(
            wlf.reshape(NCH, 128, NO).transpose(1, 0, 2)).reshape(
                128, NCH * NO)
        m = {"xf8": xf8, "wf8": wf8, "xt": xt_pm.astype(bf),
             "xik": xik_pm.astype(bf), "wl": wl_pm.astype(bf), "fmat": F}
        in_maps.append(m)
    return in_maps


def _run(in_maps, trace=False, all_cores=False):
    from concourse.bass_utils import run_bass_kernel_spmd

    if "nc" not in _CACHE:
        _CACHE["nc"] = _build_program()
    nc = _CACHE["nc"]
    kwargs = {}
    if all_cores:
        kwargs["trace_cores"] = list(range(N_CORES))
    res = run_bass_kernel_spmd(nc, in_maps, core_ids=list(range(N_CORES)),
                               trace=trace, **kwargs)
    return res


def kernel(x: np.ndarray, W: np.ndarray) -> np.ndarray:
    in_maps = _host_prep(x, W)
    res = _run(in_maps)
    v = np.concatenate([res.results[c]["y"] for c in range(N_CORES)], axis=0)
    return v.reshape(B, N_NODE, O_SZ, 1).astype(np.float32)
